# revision 2
# baseline (speedup 1.0000x reference)
"""Trainium2 Bass kernel for nn_BackAttention_16398185136493.

kernel(**inputs) takes the FULL unsharded inputs and returns the FULL
(8, 512, 48, 48) output. Internally: data-parallel over the batch dim,
one batch per NeuronCore across 8 cores; the small (N, B) out_co tensor
is AllGathered to satisfy the torch-view (N,B)->(B,N) reinterpretation.
"""



import math
from contextlib import ExitStack

import numpy as np

B, C, H, Wd = 8, 512, 48, 48
N = H * Wd  # 2304
KK = 13  # knots (degree 12 poly)
NB = N // 128  # 18
CB = C // 128  # 4
M_CH = 384
N_MCH = N // M_CH  # 6
SCALE = 1.0 / math.sqrt(C)


def cheb_knots():
    j = np.arange(KK)
    return ((1.0 - np.cos(np.pi * j / (KK - 1))) / 2.0).astype(np.float64)


def interp_matrix():
    # centered basis (s-1/2)^i: far better conditioned for fp32 eval
    kn = cheb_knots() - 0.5
    V = np.vander(kn, KK, increasing=True)
    return np.linalg.inv(V)


def n_chunks(total, ch):
    out, o = [], 0
    while o < total:
        out.append((o, min(ch, total - o)))
        o += ch
    return out


def build_nc(debug=False):
    import concourse.bass as bass
    import concourse.tile as tile
    from concourse import mybir

    dt = mybir.dt
    AF = mybir.ActivationFunctionType
    AL = mybir.AluOpType
    AX = mybir.AxisListType
    f32, f32r, bf16 = dt.float32, dt.float32r, dt.bfloat16
    knots = cheb_knots()

    nc = bass.Bass("TRN2", debug=False, num_devices=8)

    x_d = nc.dram_tensor("x", [C, N], f32r, kind="ExternalInput").ap()
    wT = {}
    for nm in ["conv", "q1", "q2", "k1", "k2", "v1", "v2", "atten"]:
        wT[nm] = nc.dram_tensor(f"{nm}_wT", [C, C], f32r, kind="ExternalInput").ap()
    bias_d = {}
    for nm in ["conv", "q1", "q2", "k1", "k2", "v1"]:
        bias_d[nm] = nc.dram_tensor(f"{nm}_b", [CB, 128, 1], f32,
                                    kind="ExternalInput").ap()
    v2b_d = nc.dram_tensor("v2_b", [1, C], f32r, kind="ExternalInput").ap()
    minv_d = nc.dram_tensor("minv", [KK, 128, KK], f32, kind="ExternalInput").ap()
    bsel_d = nc.dram_tensor("bsel", [B, 128], f32r, kind="ExternalInput").ap()
    ones_d = nc.dram_tensor("ones", [128, 128], f32r, kind="ExternalInput").ap()
    y_d = nc.dram_tensor("y", [C, N], f32, kind="ExternalOutput").ap()

    xc_dram = nc.dram_tensor("xc_dram", [C, N], f32).ap()
    out_dram = nc.dram_tensor("out_dram", [C, N], f32r).ap()
    oc_dram = nc.dram_tensor("oc_dram", [1, N], f32).ap()
    oc_ag = nc.dram_tensor("oc_ag", [B, N], f32, addr_space="Shared").ap()

    dbg = {}
    if debug:
        for nm, shp in [("xc", [C, N]), ("q", [C, N]), ("k", [C, N]),
                        ("v", [N, C]), ("ab0", [128, N]), ("W0", [128, N]),
                        ("T0", [128, KK]), ("rp10", [128, N]),
                        ("outT", [C, N]), ("oc", [1, N]), ("xco", [B, N]),
                        ("s0", [128, N]), ("coef0", [128, KK]),
                        ("misc0", [128, 4]), ("ocag", [B, N]),
                        ("xfraw", [B, N])]:
            dbg[nm] = nc.dram_tensor(f"dbg_{nm}", shp, f32,
                                     kind="ExternalOutput").ap()

    r32 = lambda ap: ap.bitcast(f32r)

    with tile.TileContext(nc) as tc, ExitStack() as st:
        tiny = st.enter_context(tc.tile_pool(name="tiny", bufs=1))
        psO = st.enter_context(tc.tile_pool(name="psO", bufs=2, space="PSUM"))

        with ExitStack() as stB:
            pqk = stB.enter_context(tc.tile_pool(name="pqk", bufs=1))
            pv = stB.enter_context(tc.tile_pool(name="pv", bufs=1))
            q_t = [pqk.tile([128, N], f32r, name=f"q{i}", tag=f"q{i}") for i in range(CB)]
            k_t = [pqk.tile([128, N], f32r, name=f"k{i}", tag=f"k{i}") for i in range(CB)]
            v_t = [pv.tile([128, C], bf16, name=f"v{i}", tag=f"v{i}") for i in range(NB)]

            # ================= stage A: linears =================
            with ExitStack() as stA:
                pa = stA.enter_context(tc.tile_pool(name="pa", bufs=1))
                pw = stA.enter_context(tc.tile_pool(name="pw", bufs=2))
                psA = stA.enter_context(
                    tc.tile_pool(name="psA", bufs=2, space="PSUM"))

                x_t = [pa.tile([128, N], f32r, name=f"x{i}", tag=f"x{i}") for i in range(CB)]
                for i in range(CB):
                    nc.sync.dma_start(x_t[i][:], x_d[i * 128:(i + 1) * 128, :])

                def load_w(nm):
                    ts = [pw.tile([128, C], f32r, name=f"w{i}", tag=f"w{i}") for i in range(CB)]
                    for i in range(CB):
                        nc.sync.dma_start(ts[i][:],
                                          wT[nm][i * 128:(i + 1) * 128, :])
                    return ts

                def load_b(nm):
                    ts = [pw.tile([128, 1], f32, name=f"b{i}", tag=f"b{i}") for i in range(CB)]
                    for i in range(CB):
                        nc.sync.dma_start(ts[i][:], bias_d[nm][i])
                    return ts

                def lin_cn(in_tiles, nm, out_tiles, act, scale=1.0):
                    w_ts, b_ts = load_w(nm), load_b(nm)
                    for co in range(CB):
                        for ns, nl in n_chunks(N, 512):
                            ps = psA.tile([128, 512], f32, name="psA", tag="psA")
                            for ci in range(CB):
                                nc.tensor.matmul(
                                    ps[:, :nl],
                                    r32(w_ts[ci][:, co * 128:(co + 1) * 128]),
                                    r32(in_tiles[ci][:, ns:ns + nl]),
                                    start=(ci == 0), stop=(ci == CB - 1))
                            nc.scalar.activation(
                                out_tiles[co][:, ns:ns + nl], ps[:, :nl], act,
                                bias=b_ts[co][:], scale=scale)

                xc_t = [pa.tile([128, N], f32r, name=f"xc{i}", tag=f"xc{i}") for i in range(CB)]
                lin_cn(x_t, "conv", xc_t, AF.Relu)
                for i in range(CB):
                    nc.sync.dma_start(xc_dram[i * 128:(i + 1) * 128, :],
                                      xc_t[i][:].bitcast(f32))
                if debug:
                    for i in range(CB):
                        nc.sync.dma_start(dbg["xc"][i * 128:(i + 1) * 128, :],
                                          xc_t[i][:].bitcast(f32))

                mid_t = [pa.tile([128, N], f32r, name=f"x{i}", tag=f"x{i}") for i in range(CB)]
                lin_cn(xc_t, "q1", mid_t, AF.Identity)
                lin_cn(mid_t, "q2", q_t, AF.Identity, scale=SCALE)
                mid_t = [pa.tile([128, N], f32r, name=f"x{i}", tag=f"x{i}") for i in range(CB)]
                lin_cn(xc_t, "k1", mid_t, AF.Identity)
                lin_cn(mid_t, "k2", k_t, AF.Identity)
                mid_t = [pa.tile([128, N], f32r, name=f"x{i}", tag=f"x{i}") for i in range(CB)]
                lin_cn(xc_t, "v1", mid_t, AF.Identity)

                vw = load_w("v2")
                v2b = pw.tile([1, C], f32r, name="v2b", tag="v2b")
                nc.sync.dma_start(v2b[:], v2b_d[:])
                ones1 = pw.tile([1, 128], f32r, name="ones1", tag="ones1")
                nc.sync.dma_start(ones1[:], ones_d[0:1, :])
                for nt in range(NB):
                    ps = psA.tile([128, 512], f32, name="psA", tag="psA")
                    for ci in range(CB):
                        nc.tensor.matmul(
                            ps[:], r32(mid_t[ci][:, nt * 128:(nt + 1) * 128]),
                            r32(vw[ci][:]), start=(ci == 0), stop=False)
                    nc.tensor.matmul(ps[:], r32(ones1[:]), r32(v2b[:]),
                                     start=False, stop=True)
                    nc.scalar.activation(v_t[nt][:], ps[:], AF.Copy)
                    if debug:
                        vtmp = pw.tile([128, C], f32, name="vtmp", tag="vtmp")
                        nc.vector.tensor_copy(vtmp[:], v_t[nt][:])
                        nc.sync.dma_start(
                            dbg["v"][nt * 128:(nt + 1) * 128, :], vtmp[:])
                if debug:
                    for i in range(CB):
                        nc.sync.dma_start(dbg["q"][i * 128:(i + 1) * 128, :],
                                          q_t[i][:].bitcast(f32))
                        nc.sync.dma_start(dbg["k"][i * 128:(i + 1) * 128, :],
                                          k_t[i][:].bitcast(f32))

            # ============ stage B/C: attention blocks ============
            with ExitStack() as stC:
                work = stC.enter_context(tc.tile_pool(name="work", bufs=1))
                dbl = stC.enter_context(tc.tile_pool(name="dbl", bufs=2))
                psB = stC.enter_context(
                    tc.tile_pool(name="psB", bufs=3, space="PSUM"))

                minv_t = tiny.tile([128, KK * KK], f32, name="minv", tag="minv")
                for i in range(KK):
                    nc.sync.dma_start(minv_t[:, i * KK:(i + 1) * KK], minv_d[i])

                act_kb = {}
                for kk in [7, 8, 9, 10, 11]:
                    kb = tiny.tile([128, 1], f32, name=f"kb{kk}", tag=f"kb{kk}")
                    nc.vector.memset(kb[:], -float(knots[kk] - 0.5))
                    act_kb[kk] = kb

                for blk in range(NB):
                    qs = slice(blk * 128, (blk + 1) * 128)
                    ab = work.tile([128, N], f32, name="ab", tag="ab")
                    for mi in range(N_MCH):
                        ms = slice(mi * M_CH, (mi + 1) * M_CH)
                        ps = psB.tile([128, M_CH], f32, name="psB", tag="psB")
                        for ci in range(CB):
                            nc.tensor.matmul(
                                ps[:], r32(q_t[ci][:, qs]), r32(k_t[ci][:, ms]),
                                start=(ci == 0), stop=(ci == CB - 1))
                        nc.scalar.activation(ab[:, ms], ps[:], AF.Copy)
                    if debug and blk == 0:
                        nc.sync.dma_start(dbg["ab0"][:], ab[:])

                    rmax_n = tiny.tile([128, 1], f32, name="rmax_n", tag="rmax_n")
                    nc.vector.tensor_reduce(rmax_n[:], ab[:], AX.X, AL.max,
                                            negate=True)
                    pos = work.tile([128, N], bf16, name="pos", tag="pos")
                    npos = tiny.tile([128, 1], f32, name="npos", tag="npos")
                    nc.vector.tensor_scalar(pos[:], ab[:], 0.0, 0.0,
                                            AL.is_ge, AL.add,
                                            accum_out=npos[:])

                    E = work.tile([128, N], f32, name="E", tag="E")
                    Z = tiny.tile([128, 1], f32, name="Z", tag="Z")
                    nc.scalar.activation(E[:], ab[:], AF.Exp, bias=rmax_n[:],
                                         scale=1.0, accum_out=Z[:])
                    rz = tiny.tile([128, 1], f32, name="rz", tag="rz")
                    nc.vector.reciprocal(rz[:], Z[:])

                    thr = tiny.tile([128, 1], f32, name="thr", tag="thr")
                    nc.scalar.activation(thr[:], rmax_n[:], AF.Exp)
                    den = tiny.tile([128, 1], f32, name="den", tag="den")
                    nc.vector.tensor_scalar(den[:], thr[:], -1.0, 1.0,
                                            AL.mult, AL.add)
                    rden = tiny.tile([128, 1], f32, name="rden", tag="rden")
                    nc.vector.reciprocal(rden[:], den[:])
                    nthr = tiny.tile([128, 1], f32, name="nthr", tag="nthr")
                    nc.vector.tensor_scalar(nthr[:], thr[:], -1.0, None,
                                            AL.mult)
                    sbias = tiny.tile([128, 1], f32, name="sbias", tag="sbias")
                    nc.vector.tensor_tensor(sbias[:], nthr[:], rden[:],
                                            AL.mult)

                    s = work.tile([128, N], f32, name="s", tag="s")
                    nc.scalar.activation(s[:], E[:], AF.Relu, bias=sbias[:],
                                         scale=rden[:])
                    nc.vector.tensor_scalar(s[:], s[:], -0.5, None, AL.add)

                    T = tiny.tile([128, 16], f32, name="T", tag="T")
                    jD = work.tile([128, N], bf16, name="jD", tag="jD")
                    jA = work.tile([128, N], bf16, name="jA", tag="jA")
                    for kk in [1, 2, 3, 4, 5, 6]:
                        nc.vector.tensor_scalar(
                            jD[:], s[:], float(knots[kk] - 0.5), 0.0,
                            AL.is_gt, AL.add, accum_out=T[:, kk:kk + 1])
                    for kk in [7, 8, 9, 10, 11]:
                        nc.scalar.activation(
                            jA[:], s[:], AF.Sign, bias=act_kb[kk][:],
                            scale=1.0, accum_out=T[:, kk:kk + 1])
                    nc.vector.tensor_scalar(T[:, 7:12], T[:, 7:12], 0.5,
                                            float(N) * 0.5, AL.mult, AL.add)
                    nc.vector.tensor_copy(T[:, 0:1], npos[:])
                    nc.vector.memset(T[:, KK - 1:KK], 0.0)
                    if debug and blk == 0:
                        Tt = tiny.tile([128, KK], f32, name="Tt", tag="Tt")
                        nc.vector.tensor_copy(Tt[:], T[:, 0:KK])
                        nc.sync.dma_start(dbg["T0"][:], Tt[:])
                        nc.sync.dma_start(dbg["s0"][:], s[:])

                    coef = tiny.tile([128, KK], f32, name="coef", tag="coef")
                    j13 = tiny.tile([128, KK], f32, name="j13", tag="j13")
                    for i in range(KK):
                        nc.vector.scalar_tensor_tensor(
                            j13[:], T[:, 0:KK], 1.0,
                            minv_t[:, i * KK:(i + 1) * KK],
                            AL.mult, AL.mult, accum_out=coef[:, i:i + 1])

                    h = work.tile([128, N], f32, name="h", tag="h")
                    nc.vector.tensor_scalar(h[:], s[:], coef[:, KK - 1:KK],
                                            None, AL.mult)
                    for i in range(KK - 2, 0, -1):
                        nc.vector.scalar_tensor_tensor(
                            h[:], h[:], coef[:, i:i + 1], s[:], AL.add,
                            AL.mult)
                    apn = tiny.tile([128, 1], f32, name="apn", tag="apn")
                    nc.vector.tensor_scalar(apn[:], npos[:], -1.0, None,
                                            AL.add)
                    if debug and blk == 0:
                        nc.sync.dma_start(dbg["coef0"][:], coef[:, 0:KK])
                        msc = tiny.tile([128, 4], f32, name="msc", tag="msc")
                        nc.vector.tensor_copy(msc[:, 0:1], npos[:])
                        nc.vector.tensor_copy(msc[:, 1:2], apn[:])
                        nc.vector.tensor_copy(msc[:, 2:3], Z[:])
                        nc.vector.tensor_copy(msc[:, 3:4], rmax_n[:])
                        nc.sync.dma_start(dbg["misc0"][:], msc[:])
                    rp1 = work.tile([128, N], f32, name="s", tag="s")
                    nc.vector.tensor_scalar(rp1[:], h[:], coef[:, 0:1], 0.0,
                                            AL.add, AL.max)
                    nc.vector.tensor_scalar(rp1[:], rp1[:], apn[:], 1.0,
                                            AL.min, AL.add)
                    if debug and blk == 0:
                        nc.sync.dma_start(dbg["rp10"][:], rp1[:])

                    sq = work.tile([128, N], f32, name="h", tag="h")
                    nc.scalar.activation(sq[:], rp1[:], AF.Square)
                    nc.vector.tensor_tensor(sq[:], sq[:], rp1[:], AL.mult)
                    nc.vector.scalar_tensor_tensor(sq[:], sq[:], -1.0, pos[:],
                                                   AL.add, AL.mult)
                    nc.vector.scalar_tensor_tensor(E[:], sq[:], 1.0, E[:],
                                                   AL.add, AL.mult)
                    Wb = dbl.tile([128, N], bf16, name="Wb", tag="Wb")
                    nc.vector.tensor_scalar(Wb[:], E[:], rz[:], None, AL.mult)
                    if debug and blk == 0:
                        Wtmp = work.tile([128, N], f32, name="ab", tag="ab")
                        nc.vector.tensor_copy(Wtmp[:], Wb[:])
                        nc.sync.dma_start(dbg["W0"][:], Wtmp[:])

                    WT = dbl.tile([128, N], bf16, name="WT", tag="WT")
                    nc.scalar.dma_start_transpose(
                        WT[:].rearrange("p (c f) -> p c f", c=NB), Wb[:])

                    for co in range(CB):
                        po = psO.tile([128, 128], f32, name="o", tag="o")
                        for mi in range(NB):
                            nc.tensor.matmul(
                                po[:], v_t[mi][:, co * 128:(co + 1) * 128],
                                WT[:, mi * 128:(mi + 1) * 128],
                                start=(mi == 0), stop=(mi == NB - 1))
                        ost = dbl.tile([128, 128], f32r, name="ost", tag="ost")
                        nc.scalar.activation(ost[:], po[:], AF.Copy)
                        nc.sync.dma_start(
                            out_dram[co * 128:(co + 1) * 128, qs], ost[:])

        # ================= stage D =================
        with ExitStack() as stD:
            pd = stD.enter_context(tc.tile_pool(name="pd", bufs=1))
            out_t = [pd.tile([128, N], f32r, name=t, tag=t) for t in
                     ("g0", "g1", "g2", "g3")]
            for i in range(CB):
                nc.sync.dma_start(out_t[i][:],
                                  out_dram[i * 128:(i + 1) * 128, :])
            if debug:
                for i in range(CB):
                    nc.sync.dma_start(dbg["outT"][i * 128:(i + 1) * 128, :],
                                      out_t[i][:].bitcast(f32))

            cmx = [pd.tile([1, N], f32, name=f"cmx{i}", tag=f"cmx{i}")
                   for i in range(CB)]
            for i in range(CB):
                nc.gpsimd.tensor_reduce(cmx[i][:], out_t[i][:].bitcast(f32), AX.C, AL.max)
            nc.vector.tensor_tensor(cmx[0][:], cmx[0][:], cmx[1][:], AL.max)
            nc.vector.tensor_tensor(cmx[2][:], cmx[2][:], cmx[3][:], AL.max)
            nc.vector.tensor_tensor(cmx[0][:], cmx[0][:], cmx[2][:], AL.max)

            ones128 = tiny.tile([128, 1], f32, name="ones128", tag="ones128")
            nc.sync.dma_start(ones128[:], ones_d[:, 0:1].bitcast(f32))
            csum = pd.tile([1, N], f32, name="csum", tag="csum")
            for ns, nl in n_chunks(N, 512):
                ps = psO.tile([1, 512], f32, name="o", tag="o")
                for ci in range(CB):
                    nc.tensor.matmul(ps[:, :nl], ones128[:],
                                     out_t[ci][:, ns:ns + nl].bitcast(f32),
                                     start=(ci == 0), stop=(ci == CB - 1))
                nc.scalar.activation(csum[:, ns:ns + nl], ps[:, :nl], AF.Copy)

            oc = pd.tile([1, N], f32, name="oc", tag="oc")
            nc.vector.scalar_tensor_tensor(oc[:], csum[:], 1.0 / C, cmx[0][:],
                                           AL.mult, AL.add)
            if debug:
                nc.sync.dma_start(dbg["oc"][:], oc[:])

            XF = pd.tile([B, N], f32, name="XF", tag="XF")
            cc_sem = nc.alloc_semaphore("cc_sem")
            with tc.tile_critical():
                nc.sync.dma_start(oc_dram[:], oc[:]).then_inc(cc_sem, 16)
                nc.gpsimd.wait_ge(cc_sem, 16)
                nc.gpsimd.collective_compute(
                    "AllGather", AL.bypass, replica_groups=[list(range(B))],
                    ins=[oc_dram[:]], outs=[oc_ag[:]]).then_inc(cc_sem, 1)
                nc.gpsimd.wait_ge(cc_sem, 17)
                nc.sync.wait_ge(cc_sem, 17)
                XF3 = XF[:].rearrange("b (f p) -> b f p", p=B)
                for p in range(B):
                    nc.sync.dma_start(
                        XF3[:, :, p],
                        oc_ag[p:p + 1, :].rearrange("o (b f) -> (o b) f", b=B)
                    ).then_inc(cc_sem, 16)
                if debug:
                    nc.sync.dma_start(dbg["ocag"][:],
                                      oc_ag[:]).then_inc(cc_sem, 16)
                nc.vector.wait_ge(cc_sem, 17 + 16 * (B + (1 if debug else 0)))

            if debug:
                nc.sync.dma_start(dbg["xfraw"][:], XF[:])
            xmx = tiny.tile([B, 1], f32, name="xmx", tag="xmx")
            nc.vector.tensor_reduce(xmx[:], XF[:], AX.X, AL.max, negate=True)
            nc.vector.tensor_scalar(xmx[:], xmx[:], SCALE, None, AL.mult)
            xz = tiny.tile([B, 1], f32, name="xz", tag="xz")
            nc.scalar.activation(XF[:], XF[:], AF.Exp, bias=xmx[:],
                                 scale=SCALE, accum_out=xz[:])
            xrz = tiny.tile([B, 1], f32, name="xrz", tag="xrz")
            nc.vector.reciprocal(xrz[:], xz[:])
            nc.vector.tensor_scalar(XF[:], XF[:], xrz[:], None, AL.mult)
            if debug:
                nc.sync.dma_start(dbg["xco"][:], XF[:])

            bsel = tiny.tile([B, 128], f32r, name="bsel", tag="bsel")
            nc.sync.dma_start(bsel[:], bsel_d[:])
            xcob = pd.tile([128, N], f32, name="xcob", tag="xcob")
            for ns, nl in n_chunks(N, 512):
                ps128 = psO.tile([128, 512], f32, name="obig", tag="obig")
                nc.tensor.matmul(ps128[:, :nl], bsel[:].bitcast(f32),
                                 XF[:, ns:ns + nl], start=True, stop=True)
                nc.scalar.activation(xcob[:, ns:ns + nl], ps128[:, :nl],
                                     AF.Copy)

            xc_t2 = [pd.tile([128, N], f32, name=t, tag=t) for t in
                     ("g0", "g1", "g2", "g3")]
            for i in range(CB):
                nc.sync.dma_start(xc_t2[i][:],
                                  xc_dram[i * 128:(i + 1) * 128, :])
            pooled = tiny.tile([128, CB], f32, name="pooled", tag="pooled")
            oimg = [pd.tile([128, N], f32, name=f"od{i}", tag=f"od{i}")
                    for i in range(CB)]
            for i in range(CB):
                nc.vector.scalar_tensor_tensor(
                    oimg[i][:], xc_t2[i][:], 1.0, xcob[:], AL.mult, AL.mult,
                    accum_out=pooled[:, i:i + 1])

            pooled_n = tiny.tile([128, CB], f32, name="pooled_n",
                                 tag="pooled_n")
            nc.vector.tensor_scalar(pooled_n[:], pooled[:], 1.0 / N, None,
                                    AL.mult)
            aw = [pd.tile([128, C], f32r, name=f"aw{i}", tag=f"aw{i}")
                  for i in range(CB)]
            for i in range(CB):
                nc.sync.dma_start(aw[i][:],
                                  wT["atten"][i * 128:(i + 1) * 128, :])
            gate = tiny.tile([128, CB], f32, name="gate", tag="gate")
            for co in range(CB):
                ps = psO.tile([128, 1], f32, name="o", tag="o")
                for ci in range(CB):
                    nc.tensor.matmul(
                        ps[:], aw[ci][:, co * 128:(co + 1) * 128].bitcast(f32),
                        pooled_n[:, ci:ci + 1],
                        start=(ci == 0), stop=(ci == CB - 1))
                nc.scalar.activation(gate[:, co:co + 1], ps[:], AF.Sigmoid)

            for i in range(CB):
                nc.vector.scalar_tensor_tensor(
                    oimg[i][:], xc_t2[i][:], gate[:, i:i + 1], oimg[i][:],
                    AL.mult, AL.add)
                nc.sync.dma_start(y_d[i * 128:(i + 1) * 128, :], oimg[i][:])

    _split_excess_waits(nc)
    return nc


def _split_excess_waits(nc, max_waits=1):
    import bass_rust
    from concourse import mybir
    for f in nc.m.functions:
        for blk in f.blocks:
            insts = list(blk.instructions)
            changed = False
            out = []
            for inst in insts:
                si = inst.sync_info
                if si is not None and si.on_wait and len(si.on_wait) > max_waits:
                    waits = list(si.on_wait)
                    k = 0
                    while len(waits) - k > max_waits:
                        nop = mybir.InstNoOp(name=f"{inst.name}_wsplit{k}",
                                             ins=[], outs=[])
                        nop.engine = inst.engine
                        nop.sync_info = bass_rust.SyncInfo(
                            on_wait=waits[k:k + max_waits], on_update=[])
                        out.append(nop)
                        k += max_waits
                    inst.sync_info = bass_rust.SyncInfo(
                        on_wait=waits[k:], on_update=list(si.on_update or []))
                    changed = True
                out.append(inst)
            if changed:
                blk.instructions = out


def host_inputs(inputs):
    """Prepare per-core input maps from the full problem inputs."""
    minv = interp_matrix()
    minv_b = np.ascontiguousarray(
        np.repeat(minv[:, None, :], 128, axis=1).astype(np.float32))
    shared = {"minv": minv_b,
              "ones": np.ones((128, 128), dtype=np.float32)}
    for nm in ["conv", "q1", "q2", "k1", "k2", "v1", "v2", "atten"]:
        w = np.asarray(inputs[f"{nm}_w"], dtype=np.float32)
        shared[f"{nm}_wT"] = np.ascontiguousarray(w.T)
    for nm in ["conv", "q1", "k1", "k2", "v1"]:
        b = np.asarray(inputs[f"{nm}_b"], dtype=np.float32)
        shared[f"{nm}_b"] = np.ascontiguousarray(b.reshape(CB, 128, 1))
    q2b = np.asarray(inputs["q2_b"], dtype=np.float32) * SCALE
    shared["q2_b"] = np.ascontiguousarray(q2b.reshape(CB, 128, 1))
    shared["v2_b"] = np.ascontiguousarray(
        np.asarray(inputs["v2_b"], dtype=np.float32).reshape(1, C))

    x = np.asarray(inputs["x"], dtype=np.float32)
    maps = []
    for b in range(B):
        m = dict(shared)
        m["x"] = np.ascontiguousarray(x[b].reshape(C, N))
        bsel = np.zeros((B, 128), dtype=np.float32)
        bsel[b, :] = 1.0
        m["bsel"] = bsel
        maps.append(m)
    return maps


_CACHE = {}


def kernel(**inputs):
    import numpy as np
    from concourse.bass_utils import run_bass_kernel_spmd

    if "nc" not in _CACHE:
        _CACHE["nc"] = build_nc(debug=False)
    nc = _CACHE["nc"]
    maps = host_inputs(inputs)
    res = run_bass_kernel_spmd(nc, maps, list(range(B)))
    _CACHE["last_res"] = res
    y = np.stack([np.asarray(res.results[b]["y"], dtype=np.float32)
                  .reshape(C, H, Wd) for b in range(B)])
    return y

